# revision 19
# baseline (speedup 1.0000x reference)
"""GCN layer (GCNConv + PReLU) on TRN2, SPMD across 8 NeuronCores.

Problem: out = PReLU(A_hat @ (x @ W) + b), A_hat = D^-1/2 (A+I) D^-1/2,
x: [100000, 128] f32, edge_index: [2, 1600000] int, W: [128,128], b,
prelu_a: [128].

Strategy (aggregation commutes with the linear map): out = PReLU((A_hat@x)@W+b).
Nodes are split into 8 contiguous ranges of 12500 (one per core); dst windows
of 128 nodes are dealt to per-core "slots" in count-sorted order so the
uniform (max-across-cores) padding stays small. Slots are processed in groups
of GW=6 (one PSUM bank each; 2 spare banks for the epilogue matmul).

The dominant memory-bound step is fetching x[src] (bf16) for every edge:
  - Self-loops need no gather: their rows are contiguous in x; the host
    stages them slot-ordered into a per-core `xself` tensor -> one plain
    HWDGE DMA per window, H = diag(dinv^2). (Keeping self-loops in the
    gather stream also skews quadrant balance badly: each core's 12500
    self-srcs share one quadrant.)
  - Other edges are split by src-quadrant (4 ranges of 25000 ids -> int16
    indices) and packed w-major per (group, quadrant) batch with NO
    per-window alignment; a tile crossing window boundaries gets one
    matmul/H "op" per (tile, window) role, with dstloc=255 masking edges
    of other roles. Batches are fetched by `dma_gather` in <=1024-row
    sub-gathers (HW SWDGE descriptor-ring limit), round-robin over 4 SWDGE
    queues (each queue runs on its own Q7 core pair; measured ~0.9ns/row
    vs ~7ns single-queue), single_packet=False.

Compute per op: H[e, j] = norm_e * (iota_j == dstloc_e) in ONE DVE
tensor_scalar, then PE accumulates accT[ch, node] += rows_tile.T @ H into
the window's PSUM bank. Epilogue per window: zT = W.T @ accT (PE), PReLU
via per-partition bias/slope tensor_scalars (DVE), bf16 result into a
resident y_sb, one final DMA. Host transposes/upcasts y ([128 ch, 12500
node] bf16 -> [12500, 128] f32).

No collectives. Host work is index/sharding prep only.
"""

import math

import ml_dtypes
import numpy as np

import concourse.bacc as bacc
import concourse.mybir as mybir
import concourse.tile as tile
from concourse.bass_utils import run_bass_kernel_spmd
from concourse.library_config import mlp

P = 128
N_CORES = 8
N_NODES = 100000
RPC = N_NODES // N_CORES  # rows per core: 12500
NW = math.ceil(RPC / P)  # windows per core: 98
GW = 6  # windows per group (one PSUM bank each; +2 banks for zT)
NQ = 4  # src quadrants
QS = N_NODES // NQ  # quadrant size: 25000
GTILES = 8  # tiles per sub-gather (1024 rows: HW ring limit)
NQUEUES = 4

F32 = mybir.dt.float32
BF16 = mybir.dt.bfloat16
I16 = mybir.dt.int16


def _build_program(plan, gather_only=False, skip_gather=False, skip_h=False, skip_mm=False, decouple=False, reps=1, pure_gather=False):
    n_ops = plan["n_ops"]
    ntile = plan["ntile"]  # total gather tiles
    groups = plan["groups"]
    # groups: list of dicts with
    #   ws: [slot...]
    #   self_ops: {w: (col, first, last)}
    #   batches: [(q, tile0, ntiles, ops)] where ops = [(tj, col, w, first, last)]

    nc = bacc.Bacc("TRN2", target_bir_lowering=False, num_swdge_queues=NQUEUES)
    x = nc.declare_dram_parameter("x", [N_NODES, P], BF16, isOutput=False)
    xs_p = nc.declare_dram_parameter("xself", [NW * P, P], BF16, isOutput=False)
    dn_p = nc.declare_dram_parameter("dn", [P, n_ops], F32, isOutput=False)
    nm_p = nc.declare_dram_parameter("nm", [P, n_ops], F32, isOutput=False)
    ix_p = nc.declare_dram_parameter("ix", [P, 8 * ntile], I16, isOutput=False)
    w_p = nc.declare_dram_parameter("W", [P, P], BF16, isOutput=False)
    b_p = nc.declare_dram_parameter("b", [P, 1], F32, isOutput=False)
    a_p = nc.declare_dram_parameter("a", [P, 1], F32, isOutput=False)
    iota_p = nc.declare_dram_parameter("iota", [P, P], BF16, isOutput=False)
    y = nc.declare_dram_parameter("y", [P, RPC], BF16, isOutput=True)

    with tile.TileContext(nc) as tc:
        nc.gpsimd.load_library(mlp)
        with (
            tc.tile_pool(name="meta", bufs=1) as meta_pool,
            tc.tile_pool(name="const", bufs=1) as const_pool,
            tc.tile_pool(name="ysb", bufs=1) as y_pool,
            tc.tile_pool(name="rows", bufs=12) as rows_pool,
            tc.tile_pool(name="selfrows", bufs=12) as self_pool,
            tc.tile_pool(name="h", bufs=16) as h_pool,
            tc.tile_pool(name="epi", bufs=4) as epi_pool,
            tc.tile_pool(name="psacc", bufs=GW, space="PSUM") as psacc_pool,
            tc.tile_pool(name="psz", bufs=2, space="PSUM") as psz_pool,
        ):
            dn_t = meta_pool.tile([P, n_ops], F32, tag="dn")
            nm_t = meta_pool.tile([P, n_ops], F32, tag="nm")
            ix_t = meta_pool.tile([P, 8 * ntile], I16, tag="ix")
            nc.sync.dma_start(out=dn_t[:], in_=dn_p[:, :])
            nc.sync.dma_start(out=nm_t[:], in_=nm_p[:, :])
            nc.sync.dma_start(out=ix_t[:], in_=ix_p[:, :])

            w_t = const_pool.tile([P, P], BF16, tag="W")
            iota_t = const_pool.tile([P, P], BF16, tag="iota")
            b_t = const_pool.tile([P, 1], F32, tag="b")
            a_t = const_pool.tile([P, 1], F32, tag="a")
            nc.sync.dma_start(out=w_t[:], in_=w_p[:, :])
            nc.sync.dma_start(out=iota_t[:], in_=iota_p[:, :])
            nc.sync.dma_start(out=b_t[:], in_=b_p[:, :])
            nc.sync.dma_start(out=a_t[:], in_=a_p[:, :])

            y_sb = y_pool.tile([P, RPC], BF16, tag="ysb")

            acc = {}

            def epilogue(w, accT):
                r0 = w * P
                nr = min(P, RPC - r0)
                accT_sb = epi_pool.tile([P, P], BF16, tag="accT_sb")
                nc.vector.tensor_copy(out=accT_sb[:], in_=accT)
                zT = psz_pool.tile([P, P], F32, tag="zT")
                nc.tensor.matmul(
                    out=zT[:], lhsT=w_t[:], rhs=accT_sb[:], start=True, stop=True
                )
                zc = epi_pool.tile([P, P], BF16, tag="zc")
                nc.vector.tensor_copy(out=zc[:], in_=zT[:])
                pos = epi_pool.tile([P, P], BF16, tag="pos")
                nc.vector.tensor_scalar(
                    out=pos[:],
                    in0=zc[:],
                    scalar1=b_t[:],
                    scalar2=0.0,
                    op0=mybir.AluOpType.add,
                    op1=mybir.AluOpType.max,
                )
                neg = epi_pool.tile([P, P], BF16, tag="neg")
                nc.vector.tensor_scalar(
                    out=neg[:],
                    in0=zc[:],
                    scalar1=b_t[:],
                    scalar2=0.0,
                    op0=mybir.AluOpType.add,
                    op1=mybir.AluOpType.min,
                )
                nega = epi_pool.tile([P, P], BF16, tag="nega")
                nc.vector.tensor_scalar(
                    out=nega[:],
                    in0=neg[:],
                    scalar1=a_t[:],
                    scalar2=None,
                    op0=mybir.AluOpType.mult,
                )
                nc.vector.tensor_tensor(
                    out=y_sb[:, r0 : r0 + nr],
                    in0=pos[:, :nr],
                    in1=nega[:, :nr],
                    op=mybir.AluOpType.add,
                )

            def emit_op(lhs_ap, col, w, first, last):
                if first and not skip_mm:
                    acc[w] = psacc_pool.tile(
                        [P, P], F32, tag="accT", name=f"accT{w % GW}"
                    )
                if not skip_h:
                    h_t = h_pool.tile([P, P], BF16, tag="h")
                    nc.vector.tensor_scalar(
                        out=h_t[:],
                        in0=iota_t[:],
                        scalar1=dn_t[:, col : col + 1],
                        scalar2=nm_t[:, col : col + 1],
                        op0=mybir.AluOpType.is_equal,
                        op1=mybir.AluOpType.mult,
                    )
                if skip_mm:
                    return
                nc.tensor.matmul(
                    out=acc[w][:],
                    lhsT=lhs_ap,
                    rhs=iota_t[:] if (skip_h or decouple) else h_t[:],
                    start=bool(first),
                    stop=bool(last),
                    skip_group_check=True,
                )
                if last:
                    epilogue(w, acc.pop(w)[:])

            qrr = 0
            for _rep in range(reps):
              for grp in groups:
                if pure_gather:
                    for q, tile0, ntiles, ops in grp["batches"]:
                        for k in range(0, ntiles, GTILES):
                            nt = min(GTILES, ntiles - k)
                            rows = rows_pool.tile([P, GTILES, P], BF16, tag="rows")
                            nc.gpsimd.dma_gather(
                                rows[:, :nt, :],
                                x[q * QS : (q + 1) * QS, :],
                                ix_t[:, 8 * (tile0 + k) : 8 * (tile0 + k + nt)],
                                nt * P,
                                nt * P,
                                P,
                                queue_num=qrr % NQUEUES,
                                single_packet=False,
                            )
                            qrr += 1
                    continue
                for w, (col, first, last) in grp["self_ops"].items():
                    srow = self_pool.tile([P, P], BF16, tag="selfrows")
                    nc.sync.dma_start(out=srow[:], in_=xs_p[w * P : (w + 1) * P, :])
                    if gather_only:
                        acc[w] = psacc_pool.tile(
                            [P, P], F32, tag="accT", name=f"accT{w % GW}"
                        )
                        nc.tensor.matmul(
                            out=acc[w][:], lhsT=srow[:], rhs=iota_t[:],
                            start=True, stop=True, skip_group_check=True,
                        )
                        epilogue(w, acc.pop(w)[:])
                        continue
                    emit_op(srow[:], col, w, first, last)
                for q, tile0, ntiles, ops in grp["batches"]:
                    # sub-gathers of <= GTILES tiles
                    subrows = []
                    for k in range(0, ntiles, GTILES):
                        nt = min(GTILES, ntiles - k)
                        if skip_gather:
                            subrows.append(None)
                            continue
                        rows = rows_pool.tile([P, GTILES, P], BF16, tag="rows")
                        nc.gpsimd.dma_gather(
                            rows[:, :nt, :],
                            x[q * QS : (q + 1) * QS, :],
                            ix_t[:, 8 * (tile0 + k) : 8 * (tile0 + k + nt)],
                            nt * P,
                            nt * P,
                            P,
                            queue_num=qrr % NQUEUES,
                            single_packet=False,
                        )
                        qrr += 1
                        subrows.append(rows)
                    for tj, col, w, first, last in ops:
                        if gather_only:
                            if last:
                                epilogue(w, acc.pop(w)[:]) if w in acc else None
                            continue
                        sr = subrows[tj // GTILES]
                        lhs = iota_t[:] if sr is None else sr[:, tj % GTILES, :]
                        emit_op(lhs, col, w, first, last)

              if pure_gather:
                  nc.vector.memset(y_sb[:, :P], 0.0)
              nc.sync.dma_start(out=y[:, :P], in_=y_sb[:, :P]) if pure_gather else nc.sync.dma_start(out=y[:, :], in_=y_sb[:])
    nc.compile()
    return nc


def _preprocess(x, edge_index, n_cores=N_CORES, aligned=False):
    N = x.shape[0]
    src_e = np.asarray(edge_index[0], dtype=np.int64)
    dst_e = np.asarray(edge_index[1], dtype=np.int64)
    # degree includes self-loops (A+I)
    deg = np.bincount(dst_e, minlength=N) + 1
    dinv = (1.0 / np.sqrt(deg.astype(np.float64))).astype(np.float32)
    norm_e = dinv[src_e] * dinv[dst_e]

    order = np.argsort(dst_e, kind="stable")
    src = src_e[order]
    dst = dst_e[order]
    norm = norm_e[order]

    core = dst // RPC
    local = dst - core * RPC
    win = local // P
    dstloc = (local % P).astype(np.float32)

    counts_w = np.bincount(core * NW + win, minlength=n_cores * NW).reshape(
        n_cores, NW
    )
    perm = np.empty((n_cores, NW), dtype=np.int64)  # perm[c, slot] = window
    for c in range(n_cores):
        perm[c, : NW - 1] = np.argsort(-counts_w[c, : NW - 1], kind="stable")
        perm[c, NW - 1] = NW - 1
    inv_perm = np.empty_like(perm)
    np.put_along_axis(inv_perm, perm, np.arange(NW)[None, :], axis=1)
    slot = inv_perm[core, win]

    q = src // QS
    idxl = (src - q * QS).astype(np.int16)

    # per (core, slot, q) counts and within-group w-major offsets
    key = (core * NW + slot) * NQ + q
    cnt = np.bincount(key, minlength=n_cores * NW * NQ).reshape(n_cores, NW, NQ)

    n_groups = math.ceil(NW / GW)
    # batch sizes: per (group, q): tiles = ceil(max_core sum_w cnt / P)
    groups = []
    ntile = 0
    n_ops = 0
    # edge position computation pieces
    # rank of edge within (core, slot, q)
    ord2 = np.argsort(key, kind="stable")
    gstart = np.zeros(n_cores * NW * NQ, dtype=np.int64)
    np.cumsum(cnt.ravel()[:-1], out=gstart[1:])
    rank = np.empty(len(key), dtype=np.int64)
    rank[ord2] = np.arange(len(key)) - gstart[key[ord2]]

    # offset of slot w's run within (core, group, q) batch fill
    off_wq = np.zeros((n_cores, NW, NQ), dtype=np.int64)
    for g in range(n_groups):
        ws = list(range(g * GW, min((g + 1) * GW, NW)))
        run = np.zeros((n_cores, NQ), dtype=np.int64)
        for w in ws:
            off_wq[:, w, :] = run
            run += cnt[:, w, :]

    # global position of each edge in the gather stream:
    # batch (g, q) occupies tiles [tile0, tile0+ntiles), edge pos =
    # tile0*P + off_wq[c, w, q] + rank
    tile0_gq = np.zeros((n_groups, NQ), dtype=np.int64)
    ntiles_gq = np.zeros((n_groups, NQ), dtype=np.int64)
    tstart_wq = np.zeros((NW, NQ), dtype=np.int64)
    t = 0
    for g in range(n_groups):
        ws = list(range(g * GW, min((g + 1) * GW, NW)))
        for qq in range(NQ):
            tile0_gq[g, qq] = t
            if aligned:
                for w in ws:
                    tstart_wq[w, qq] = t
                    t += max(1, -(-int(cnt[:, w, qq].max()) // P))
            else:
                fill = cnt[:, ws, qq].sum(axis=1)
                t += max(1, -(-int(fill.max()) // P))
            ntiles_gq[g, qq] = t - tile0_gq[g, qq]
    ntile = t

    if aligned:
        gslot = tstart_wq[slot, q] * P + rank
    else:
        gslot = tile0_gq[slot // GW, q] * P + off_wq[core, slot, q] + rank

    # ---- op list (uniform across cores) ----
    # For batch (g,q), tile tj: roles = union over cores of windows whose
    # [off, off+cnt) range intersects [tj*P, (tj+1)*P).
    op_col = {}  # (kind, ...) -> column
    n_ops = 0
    # window op chains: self op first, then gather ops in (q, tile) order
    chain = {w: [] for w in range(NW)}  # list of op descriptors
    for g in range(n_groups):
        ws = list(range(g * GW, min((g + 1) * GW, NW)))
        for w in ws:
            chain[w].append(("self", w))
        for qq in range(NQ):
            B = int(ntiles_gq[g, qq])
            for tj in range(B):
                if aligned:
                    tglob = int(tile0_gq[g, qq]) + tj
                    for w in ws:
                        t0w = int(tstart_wq[w, qq])
                        t1w = t0w + max(1, -(-int(cnt[:, w, qq].max()) // P))
                        if t0w <= tglob < t1w:
                            chain[w].append(("g", g, qq, tj))
                    continue
                lo, hi = tj * P, (tj + 1) * P
                for w in ws:
                    o = off_wq[:, w, qq]
                    c_ = cnt[:, w, qq]
                    if bool(np.any((o < hi) & (o + c_ > lo))):
                        chain[w].append(("g", g, qq, tj))

    # assign columns in emission order (groups -> self ops -> batches)
    groups = []
    col_of = {}
    col = 0
    for g in range(n_groups):
        ws = list(range(g * GW, min((g + 1) * GW, NW)))
        grp = {"ws": ws, "self_ops": {}, "batches": []}
        for w in ws:
            col_of[("self", w)] = col
            first = chain[w][0] == ("self", w)
            last = len(chain[w]) == 1
            grp["self_ops"][w] = (col, first, last)
            col += 1
        for qq in range(NQ):
            B = int(ntiles_gq[g, qq])
            ops = []
            for tj in range(B):
                for w in ws:
                    keyo = ("g", g, qq, tj)
                    if keyo in chain[w]:
                        col_of[("g", g, qq, tj, w)] = col
                        first = chain[w][0] == keyo
                        last = chain[w][-1] == keyo
                        ops.append((tj, col, w, first, last))
                        col += 1
            grp["batches"].append((qq, int(tile0_gq[g, qq]), B, ops))
        groups.append(grp)
    n_ops = col

    # ---- per-core planes ----
    p_of = (gslot % P).astype(np.int64)
    # op column for each edge: ("g", g, q, tile_local, w)
    g_of = slot // GW
    tloc = gslot // P - tile0_gq[g_of, q]
    edge_col = np.empty(len(key), dtype=np.int64)
    # vectorized map via dict lookup in chunks (python loop over unique keys)
    ek = list(zip(g_of.tolist(), q.tolist(), tloc.tolist(), slot.tolist()))
    edge_col = np.fromiter(
        (col_of[("g", g_, q_, t_, w_)] for (g_, q_, t_, w_) in ek),
        dtype=np.int64,
        count=len(ek),
    )

    ix_col = 8 * (gslot // P) + (gslot % P) // 16
    ix_row = (gslot % P) % 16

    metas = []
    for c in range(n_cores):
        m = core == c
        dn_plane = np.full((P, n_ops), 255.0, dtype=np.float32)
        nm_plane = np.zeros((P, n_ops), dtype=np.float32)
        dn_plane[p_of[m], edge_col[m]] = dstloc[m]
        nm_plane[p_of[m], edge_col[m]] = norm[m]
        # self ops: dn = 0..nr-1, nm = dinv^2 of the window's nodes
        for s in range(NW):
            colw = col_of[("self", s)]
            wreal = int(perm[c, s])
            nr = min(P, RPC - wreal * P)
            rows0 = c * RPC + wreal * P
            dn_plane[:nr, colw] = np.arange(nr, dtype=np.float32)
            nm_plane[:nr, colw] = dinv[rows0 : rows0 + nr] ** 2
        ix_plane = np.zeros((16, 8 * ntile), dtype=np.int16)
        ix_plane[ix_row[m], ix_col[m]] = idxl[m]
        ix_full = np.tile(ix_plane, (8, 1))
        metas.append(
            {
                "dn": dn_plane,
                "nm": nm_plane,
                "ix": np.ascontiguousarray(ix_full),
            }
        )

    plan = {"n_ops": n_ops, "ntile": ntile, "groups": groups}
    return metas, plan, perm, dinv


def _make_in_maps(x, W, b, prelu_a, metas, perm, dinv):
    xbf = np.asarray(x, dtype=np.float32).astype(ml_dtypes.bfloat16)
    consts = {
        "x": xbf,
        "W": np.asarray(W, dtype=np.float32).astype(ml_dtypes.bfloat16),
        "b": np.asarray(b, dtype=np.float32).reshape(P, 1),
        "a": np.asarray(prelu_a, dtype=np.float32).reshape(P, 1),
        "iota": np.tile(
            np.arange(P, dtype=np.float32).astype(ml_dtypes.bfloat16), (P, 1)
        ),
    }
    maps = []
    for c in range(N_CORES):
        xs = np.zeros((NW * P, P), dtype=ml_dtypes.bfloat16)
        for s in range(NW):
            wreal = int(perm[c, s])
            nr = min(P, RPC - wreal * P)
            rows0 = c * RPC + wreal * P
            xs[s * P : s * P + nr] = xbf[rows0 : rows0 + nr]
        maps.append({**consts, **metas[c], "xself": xs})
    return maps


def _unscramble(y_all, perm):
    """y_all: [n_cores*P, RPC] bf16 (ch-major, slot-ordered columns) ->
    [N_NODES, P] f32 in natural node order."""
    out = np.empty((N_CORES * RPC, P), dtype=np.float32)
    for c in range(N_CORES):
        yc = np.asarray(y_all[c * P : (c + 1) * P]).astype(np.float32).T  # [RPC, P]
        oc = out[c * RPC : (c + 1) * RPC]
        for s in range(NW):
            w = int(perm[c, s])
            nr = min(P, RPC - w * P)
            oc[w * P : w * P + nr] = yc[s * P : s * P + nr]
    return out


ALIGNED = True


def build_all(x, edge_index, W, b, prelu_a):
    """Preprocess + build. Returns (nc, in_maps, rows_per_core, unscramble)."""
    metas, plan, perm, dinv = _preprocess(x, edge_index, aligned=ALIGNED)
    nc = _build_program(plan)
    in_maps = _make_in_maps(x, W, b, prelu_a, metas, perm, dinv)
    unscramble = lambda y: _unscramble(y, perm)
    return nc, in_maps, RPC, unscramble


def kernel(x, edge_index, W, b, prelu_a):
    nc, in_maps, _, unscramble = build_all(x, edge_index, W, b, prelu_a)
    res = run_bass_kernel_spmd(nc, in_maps, core_ids=list(range(N_CORES)))
    y = np.concatenate(
        [np.asarray(res.results[c]["y"]) for c in range(N_CORES)], axis=0
    )
    return unscramble(y)


# revision 21
# speedup vs baseline: 1.7998x; 1.7998x over previous
"""GCN layer (GCNConv + PReLU) on TRN2, SPMD across 8 NeuronCores.

Problem: out = PReLU(A_hat @ (x @ W) + b), A_hat = D^-1/2 (A+I) D^-1/2,
x: [100000, 128] f32, edge_index: [2, 1600000] int, W: [128,128], b,
prelu_a: [128].

Strategy (aggregation commutes with the linear map): out = PReLU((A_hat@x)@W+b).
Nodes are split into 8 contiguous ranges of 12500 (one per core); dst windows
of 128 nodes are dealt to per-core "slots" in count-sorted order so the
uniform (max-across-cores) padding stays small. Slots are processed in groups
of GW=6 (one PSUM bank each; 2 spare banks for the epilogue matmul).

The dominant memory-bound step is fetching x[src] (bf16) for every edge:
  - Self-loops need no gather: their rows are contiguous in x; the host
    stages them slot-ordered into a per-core `xself` tensor -> one plain
    HWDGE DMA per window, H = diag(dinv^2). (Keeping self-loops in the
    gather stream also skews quadrant balance badly: each core's 12500
    self-srcs share one quadrant.)
  - Other edges are split by src-quadrant (4 ranges of 25000 ids -> int16
    indices). Default plan (ALIGNED=True): tiles aligned per (window,
    quadrant), one op per tile (~2035 ops, ~1937 tiles). The multi-role
    packed plan (aligned=False: fewer tiles, +300 ops) measured ~0.5ms
    SLOWER on HW -- DVE/PE op count dominates gather bytes here.
    Batches are fetched by `dma_gather` in <=1024-row
    sub-gathers (HW SWDGE descriptor-ring limit), round-robin over 4 SWDGE
    queues (each queue runs on its own Q7 core pair; measured ~0.9ns/row
    vs ~7ns single-queue), single_packet=False.

Compute per op: H[e, j] = norm_e * (iota_j == dstloc_e) in ONE DVE
tensor_scalar, then PE accumulates accT[ch, node] += rows_tile.T @ H into
the window's PSUM bank. Epilogue per window: zT = W.T @ accT (PE), PReLU
via per-partition bias/slope tensor_scalars (DVE), bf16 result into a
resident y_sb, one final DMA. Host transposes/upcasts y ([128 ch, 12500
node] bf16 -> [12500, 128] f32).

No collectives. Host work is index/sharding prep only.
"""

import math

import ml_dtypes
import numpy as np

import concourse.bacc as bacc
import concourse.mybir as mybir
import concourse.tile as tile
from concourse.bass_utils import run_bass_kernel_spmd
from concourse.library_config import mlp

P = 128
N_CORES = 8
N_NODES = 100000
RPC = N_NODES // N_CORES  # rows per core: 12500
NW = math.ceil(RPC / P)  # windows per core: 98
GW = 6  # windows per group (one PSUM bank each; +2 banks for zT)
NQ = 4  # src quadrants
QS = N_NODES // NQ  # quadrant size: 25000
GTILES = 8  # tiles per sub-gather (1024 rows: HW ring limit)
NQUEUES = 4

F32 = mybir.dt.float32
BF16 = mybir.dt.bfloat16
I16 = mybir.dt.int16


def _build_program(plan, gather_only=False, skip_gather=False, skip_h=False, skip_mm=False, decouple=False, reps=1, pure_gather=False):
    n_ops = plan["n_ops"]
    ntile = plan["ntile"]  # total gather tiles
    groups = plan["groups"]
    # groups: list of dicts with
    #   ws: [slot...]
    #   self_ops: {w: (col, first, last)}
    #   batches: [(q, tile0, ntiles, ops)] where ops = [(tj, col, w, first, last)]

    nc = bacc.Bacc("TRN2", target_bir_lowering=False, num_swdge_queues=NQUEUES)
    x = nc.declare_dram_parameter("x", [N_NODES, P], BF16, isOutput=False)
    xs_p = nc.declare_dram_parameter("xself", [NW * P, P], BF16, isOutput=False)
    dn_p = nc.declare_dram_parameter("dn", [P, n_ops], F32, isOutput=False)
    nm_p = nc.declare_dram_parameter("nm", [P, n_ops], F32, isOutput=False)
    ix_p = nc.declare_dram_parameter("ix", [P, 8 * ntile], I16, isOutput=False)
    w_p = nc.declare_dram_parameter("W", [P, P], BF16, isOutput=False)
    b_p = nc.declare_dram_parameter("b", [P, 1], F32, isOutput=False)
    a_p = nc.declare_dram_parameter("a", [P, 1], F32, isOutput=False)
    nb_p = nc.declare_dram_parameter("nb", [P, 1], F32, isOutput=False)
    na_p = nc.declare_dram_parameter("na", [P, 1], F32, isOutput=False)
    iota_p = nc.declare_dram_parameter("iota", [P, P], BF16, isOutput=False)
    y = nc.declare_dram_parameter("y", [P, RPC], BF16, isOutput=True)

    with tile.TileContext(nc) as tc:
        nc.gpsimd.load_library(mlp)
        with (
            tc.tile_pool(name="meta", bufs=1) as meta_pool,
            tc.tile_pool(name="const", bufs=1) as const_pool,
            tc.tile_pool(name="ysb", bufs=1) as y_pool,
            tc.tile_pool(name="rows", bufs=12) as rows_pool,
            tc.tile_pool(name="selfrows", bufs=12) as self_pool,
            tc.tile_pool(name="h", bufs=16) as h_pool,
            tc.tile_pool(name="epi", bufs=4) as epi_pool,
            tc.tile_pool(name="psacc", bufs=GW, space="PSUM") as psacc_pool,
            tc.tile_pool(name="psz", bufs=2, space="PSUM") as psz_pool,
        ):
            dn_t = meta_pool.tile([P, n_ops], F32, tag="dn")
            nm_t = meta_pool.tile([P, n_ops], F32, tag="nm")
            ix_t = meta_pool.tile([P, 8 * ntile], I16, tag="ix")
            nc.sync.dma_start(out=dn_t[:], in_=dn_p[:, :])
            nc.sync.dma_start(out=nm_t[:], in_=nm_p[:, :])
            nc.sync.dma_start(out=ix_t[:], in_=ix_p[:, :])

            w_t = const_pool.tile([P, P], BF16, tag="W")
            iota_t = const_pool.tile([P, P], BF16, tag="iota")
            b_t = const_pool.tile([P, 1], F32, tag="b")
            a_t = const_pool.tile([P, 1], F32, tag="a")
            nb_t = const_pool.tile([P, 1], F32, tag="nb")
            na_t = const_pool.tile([P, 1], F32, tag="na")
            nc.sync.dma_start(out=w_t[:], in_=w_p[:, :])
            nc.sync.dma_start(out=iota_t[:], in_=iota_p[:, :])
            nc.sync.dma_start(out=b_t[:], in_=b_p[:, :])
            nc.sync.dma_start(out=a_t[:], in_=a_p[:, :])
            nc.sync.dma_start(out=nb_t[:], in_=nb_p[:, :])
            nc.sync.dma_start(out=na_t[:], in_=na_p[:, :])

            y_sb = y_pool.tile([P, RPC], BF16, tag="ysb")

            acc = {}

            def epilogue(w, accT):
                r0 = w * P
                nr = min(P, RPC - r0)
                accT_sb = epi_pool.tile([P, P], BF16, tag="accT_sb")
                nc.vector.tensor_copy(out=accT_sb[:], in_=accT)
                zT = psz_pool.tile([P, P], F32, tag="zT")
                nc.tensor.matmul(
                    out=zT[:], lhsT=w_t[:], rhs=accT_sb[:], start=True, stop=True
                )
                # PReLU on the (otherwise idle) ACT engine:
                # pos = Relu(z + b); neg = Relu(-(z + b)) = -min(z+b, 0);
                # negs = neg * (-a)  ->  y = pos + negs. DVE only does the add.
                pos = epi_pool.tile([P, P], BF16, tag="pos")
                nc.scalar.activation(
                    out=pos[:], in_=zT[:],
                    func=mybir.ActivationFunctionType.Relu,
                    bias=b_t[:], scale=1.0,
                )
                neg = epi_pool.tile([P, P], BF16, tag="neg")
                nc.scalar.activation(
                    out=neg[:], in_=zT[:],
                    func=mybir.ActivationFunctionType.Relu,
                    bias=nb_t[:], scale=-1.0,
                )
                negs = epi_pool.tile([P, P], BF16, tag="negs")
                nc.scalar.activation(
                    out=negs[:], in_=neg[:],
                    func=mybir.ActivationFunctionType.Copy,
                    bias=0.0, scale=na_t[:],
                )
                nc.vector.tensor_tensor(
                    out=y_sb[:, r0 : r0 + nr],
                    in0=pos[:, :nr],
                    in1=negs[:, :nr],
                    op=mybir.AluOpType.add,
                )

            def emit_op(lhs_ap, col, w, first, last):
                if first and not skip_mm:
                    acc[w] = psacc_pool.tile(
                        [P, P], F32, tag="accT", name=f"accT{w % GW}"
                    )
                if not skip_h:
                    h_t = h_pool.tile([P, P], BF16, tag="h")
                    nc.vector.tensor_scalar(
                        out=h_t[:],
                        in0=iota_t[:],
                        scalar1=dn_t[:, col : col + 1],
                        scalar2=nm_t[:, col : col + 1],
                        op0=mybir.AluOpType.is_equal,
                        op1=mybir.AluOpType.mult,
                    )
                if skip_mm:
                    return
                nc.tensor.matmul(
                    out=acc[w][:],
                    lhsT=lhs_ap,
                    rhs=iota_t[:] if (skip_h or decouple) else h_t[:],
                    start=bool(first),
                    stop=bool(last),
                    skip_group_check=True,
                )
                if last:
                    epilogue(w, acc.pop(w)[:])

            qrr = 0
            for _rep in range(reps):
              for grp in groups:
                if pure_gather:
                    for q, tile0, ntiles, ops in grp["batches"]:
                        for k in range(0, ntiles, GTILES):
                            nt = min(GTILES, ntiles - k)
                            rows = rows_pool.tile([P, GTILES, P], BF16, tag="rows")
                            nc.gpsimd.dma_gather(
                                rows[:, :nt, :],
                                x[q * QS : (q + 1) * QS, :],
                                ix_t[:, 8 * (tile0 + k) : 8 * (tile0 + k + nt)],
                                nt * P,
                                nt * P,
                                P,
                                queue_num=qrr % NQUEUES,
                                single_packet=False,
                            )
                            qrr += 1
                    continue
                for w, (col, first, last) in grp["self_ops"].items():
                    srow = self_pool.tile([P, P], BF16, tag="selfrows")
                    nc.sync.dma_start(out=srow[:], in_=xs_p[w * P : (w + 1) * P, :])
                    if gather_only:
                        acc[w] = psacc_pool.tile(
                            [P, P], F32, tag="accT", name=f"accT{w % GW}"
                        )
                        nc.tensor.matmul(
                            out=acc[w][:], lhsT=srow[:], rhs=iota_t[:],
                            start=True, stop=True, skip_group_check=True,
                        )
                        epilogue(w, acc.pop(w)[:])
                        continue
                    emit_op(srow[:], col, w, first, last)
                for q, tile0, ntiles, ops in grp["batches"]:
                    # sub-gathers of <= GTILES tiles
                    subrows = []
                    for k in range(0, ntiles, GTILES):
                        nt = min(GTILES, ntiles - k)
                        if skip_gather:
                            subrows.append(None)
                            continue
                        rows = rows_pool.tile([P, GTILES, P], BF16, tag="rows")
                        nc.gpsimd.dma_gather(
                            rows[:, :nt, :],
                            x[q * QS : (q + 1) * QS, :],
                            ix_t[:, 8 * (tile0 + k) : 8 * (tile0 + k + nt)],
                            nt * P,
                            nt * P,
                            P,
                            queue_num=qrr % NQUEUES,
                            single_packet=False,
                        )
                        qrr += 1
                        subrows.append(rows)
                    for tj, col, w, first, last in ops:
                        if gather_only:
                            if last:
                                epilogue(w, acc.pop(w)[:]) if w in acc else None
                            continue
                        sr = subrows[tj // GTILES]
                        lhs = iota_t[:] if sr is None else sr[:, tj % GTILES, :]
                        emit_op(lhs, col, w, first, last)

              if pure_gather:
                  nc.vector.memset(y_sb[:, :P], 0.0)
              nc.sync.dma_start(out=y[:, :P], in_=y_sb[:, :P]) if pure_gather else nc.sync.dma_start(out=y[:, :], in_=y_sb[:])
    nc.compile()
    return nc


def _preprocess(x, edge_index, n_cores=N_CORES, aligned=False):
    N = x.shape[0]
    src_e = np.asarray(edge_index[0], dtype=np.int64)
    dst_e = np.asarray(edge_index[1], dtype=np.int64)
    # degree includes self-loops (A+I)
    deg = np.bincount(dst_e, minlength=N) + 1
    dinv = (1.0 / np.sqrt(deg.astype(np.float64))).astype(np.float32)
    norm_e = dinv[src_e] * dinv[dst_e]

    order = np.argsort(dst_e, kind="stable")
    src = src_e[order]
    dst = dst_e[order]
    norm = norm_e[order]

    core = dst // RPC
    local = dst - core * RPC
    win = local // P
    dstloc = (local % P).astype(np.float32)

    counts_w = np.bincount(core * NW + win, minlength=n_cores * NW).reshape(
        n_cores, NW
    )
    perm = np.empty((n_cores, NW), dtype=np.int64)  # perm[c, slot] = window
    for c in range(n_cores):
        perm[c, : NW - 1] = np.argsort(-counts_w[c, : NW - 1], kind="stable")
        perm[c, NW - 1] = NW - 1
    inv_perm = np.empty_like(perm)
    np.put_along_axis(inv_perm, perm, np.arange(NW)[None, :], axis=1)
    slot = inv_perm[core, win]

    q = src // QS
    idxl = (src - q * QS).astype(np.int16)

    # per (core, slot, q) counts and within-group w-major offsets
    key = (core * NW + slot) * NQ + q
    cnt = np.bincount(key, minlength=n_cores * NW * NQ).reshape(n_cores, NW, NQ)

    n_groups = math.ceil(NW / GW)
    # batch sizes: per (group, q): tiles = ceil(max_core sum_w cnt / P)
    groups = []
    ntile = 0
    n_ops = 0
    # edge position computation pieces
    # rank of edge within (core, slot, q)
    ord2 = np.argsort(key, kind="stable")
    gstart = np.zeros(n_cores * NW * NQ, dtype=np.int64)
    np.cumsum(cnt.ravel()[:-1], out=gstart[1:])
    rank = np.empty(len(key), dtype=np.int64)
    rank[ord2] = np.arange(len(key)) - gstart[key[ord2]]

    # offset of slot w's run within (core, group, q) batch fill
    off_wq = np.zeros((n_cores, NW, NQ), dtype=np.int64)
    for g in range(n_groups):
        ws = list(range(g * GW, min((g + 1) * GW, NW)))
        run = np.zeros((n_cores, NQ), dtype=np.int64)
        for w in ws:
            off_wq[:, w, :] = run
            run += cnt[:, w, :]

    # global position of each edge in the gather stream:
    # batch (g, q) occupies tiles [tile0, tile0+ntiles), edge pos =
    # tile0*P + off_wq[c, w, q] + rank
    tile0_gq = np.zeros((n_groups, NQ), dtype=np.int64)
    ntiles_gq = np.zeros((n_groups, NQ), dtype=np.int64)
    tstart_wq = np.zeros((NW, NQ), dtype=np.int64)
    t = 0
    for g in range(n_groups):
        ws = list(range(g * GW, min((g + 1) * GW, NW)))
        for qq in range(NQ):
            tile0_gq[g, qq] = t
            if aligned:
                for w in ws:
                    tstart_wq[w, qq] = t
                    t += max(1, -(-int(cnt[:, w, qq].max()) // P))
            else:
                fill = cnt[:, ws, qq].sum(axis=1)
                t += max(1, -(-int(fill.max()) // P))
            ntiles_gq[g, qq] = t - tile0_gq[g, qq]
    ntile = t

    if aligned:
        gslot = tstart_wq[slot, q] * P + rank
    else:
        gslot = tile0_gq[slot // GW, q] * P + off_wq[core, slot, q] + rank

    # ---- op list (uniform across cores) ----
    # For batch (g,q), tile tj: roles = union over cores of windows whose
    # [off, off+cnt) range intersects [tj*P, (tj+1)*P).
    op_col = {}  # (kind, ...) -> column
    n_ops = 0
    # window op chains: self op first, then gather ops in (q, tile) order
    chain = {w: [] for w in range(NW)}  # list of op descriptors
    for g in range(n_groups):
        ws = list(range(g * GW, min((g + 1) * GW, NW)))
        for w in ws:
            chain[w].append(("self", w))
        for qq in range(NQ):
            B = int(ntiles_gq[g, qq])
            for tj in range(B):
                if aligned:
                    tglob = int(tile0_gq[g, qq]) + tj
                    for w in ws:
                        t0w = int(tstart_wq[w, qq])
                        t1w = t0w + max(1, -(-int(cnt[:, w, qq].max()) // P))
                        if t0w <= tglob < t1w:
                            chain[w].append(("g", g, qq, tj))
                    continue
                lo, hi = tj * P, (tj + 1) * P
                for w in ws:
                    o = off_wq[:, w, qq]
                    c_ = cnt[:, w, qq]
                    if bool(np.any((o < hi) & (o + c_ > lo))):
                        chain[w].append(("g", g, qq, tj))

    # assign columns in emission order (groups -> self ops -> batches)
    groups = []
    col_of = {}
    col = 0
    for g in range(n_groups):
        ws = list(range(g * GW, min((g + 1) * GW, NW)))
        grp = {"ws": ws, "self_ops": {}, "batches": []}
        for w in ws:
            col_of[("self", w)] = col
            first = chain[w][0] == ("self", w)
            last = len(chain[w]) == 1
            grp["self_ops"][w] = (col, first, last)
            col += 1
        for qq in range(NQ):
            B = int(ntiles_gq[g, qq])
            ops = []
            for tj in range(B):
                for w in ws:
                    keyo = ("g", g, qq, tj)
                    if keyo in chain[w]:
                        col_of[("g", g, qq, tj, w)] = col
                        first = chain[w][0] == keyo
                        last = chain[w][-1] == keyo
                        ops.append((tj, col, w, first, last))
                        col += 1
            grp["batches"].append((qq, int(tile0_gq[g, qq]), B, ops))
        groups.append(grp)
    n_ops = col

    # ---- per-core planes ----
    p_of = (gslot % P).astype(np.int64)
    # op column for each edge: ("g", g, q, tile_local, w)
    g_of = slot // GW
    tloc = gslot // P - tile0_gq[g_of, q]
    edge_col = np.empty(len(key), dtype=np.int64)
    # vectorized map via dict lookup in chunks (python loop over unique keys)
    ek = list(zip(g_of.tolist(), q.tolist(), tloc.tolist(), slot.tolist()))
    edge_col = np.fromiter(
        (col_of[("g", g_, q_, t_, w_)] for (g_, q_, t_, w_) in ek),
        dtype=np.int64,
        count=len(ek),
    )

    ix_col = 8 * (gslot // P) + (gslot % P) // 16
    ix_row = (gslot % P) % 16

    metas = []
    for c in range(n_cores):
        m = core == c
        dn_plane = np.full((P, n_ops), 255.0, dtype=np.float32)
        nm_plane = np.zeros((P, n_ops), dtype=np.float32)
        dn_plane[p_of[m], edge_col[m]] = dstloc[m]
        nm_plane[p_of[m], edge_col[m]] = norm[m]
        # self ops: dn = 0..nr-1, nm = dinv^2 of the window's nodes
        for s in range(NW):
            colw = col_of[("self", s)]
            wreal = int(perm[c, s])
            nr = min(P, RPC - wreal * P)
            rows0 = c * RPC + wreal * P
            dn_plane[:nr, colw] = np.arange(nr, dtype=np.float32)
            nm_plane[:nr, colw] = dinv[rows0 : rows0 + nr] ** 2
        ix_plane = np.zeros((16, 8 * ntile), dtype=np.int16)
        ix_plane[ix_row[m], ix_col[m]] = idxl[m]
        ix_full = np.tile(ix_plane, (8, 1))
        metas.append(
            {
                "dn": dn_plane,
                "nm": nm_plane,
                "ix": np.ascontiguousarray(ix_full),
            }
        )

    plan = {"n_ops": n_ops, "ntile": ntile, "groups": groups}
    return metas, plan, perm, dinv


def _make_in_maps(x, W, b, prelu_a, metas, perm, dinv):
    xbf = np.asarray(x, dtype=np.float32).astype(ml_dtypes.bfloat16)
    consts = {
        "x": xbf,
        "W": np.asarray(W, dtype=np.float32).astype(ml_dtypes.bfloat16),
        "b": np.asarray(b, dtype=np.float32).reshape(P, 1),
        "a": np.asarray(prelu_a, dtype=np.float32).reshape(P, 1),
        "nb": -np.asarray(b, dtype=np.float32).reshape(P, 1),
        "na": -np.asarray(prelu_a, dtype=np.float32).reshape(P, 1),
        "iota": np.tile(
            np.arange(P, dtype=np.float32).astype(ml_dtypes.bfloat16), (P, 1)
        ),
    }
    maps = []
    for c in range(N_CORES):
        xs = np.zeros((NW * P, P), dtype=ml_dtypes.bfloat16)
        for s in range(NW):
            wreal = int(perm[c, s])
            nr = min(P, RPC - wreal * P)
            rows0 = c * RPC + wreal * P
            xs[s * P : s * P + nr] = xbf[rows0 : rows0 + nr]
        maps.append({**consts, **metas[c], "xself": xs})
    return maps


def _unscramble(y_all, perm):
    """y_all: [n_cores*P, RPC] bf16 (ch-major, slot-ordered columns) ->
    [N_NODES, P] f32 in natural node order."""
    out = np.empty((N_CORES * RPC, P), dtype=np.float32)
    for c in range(N_CORES):
        yc = np.asarray(y_all[c * P : (c + 1) * P]).astype(np.float32).T  # [RPC, P]
        oc = out[c * RPC : (c + 1) * RPC]
        for s in range(NW):
            w = int(perm[c, s])
            nr = min(P, RPC - w * P)
            oc[w * P : w * P + nr] = yc[s * P : s * P + nr]
    return out


ALIGNED = True


def build_all(x, edge_index, W, b, prelu_a):
    """Preprocess + build. Returns (nc, in_maps, rows_per_core, unscramble)."""
    metas, plan, perm, dinv = _preprocess(x, edge_index, aligned=ALIGNED)
    nc = _build_program(plan)
    in_maps = _make_in_maps(x, W, b, prelu_a, metas, perm, dinv)
    unscramble = lambda y: _unscramble(y, perm)
    return nc, in_maps, RPC, unscramble


def kernel(x, edge_index, W, b, prelu_a):
    nc, in_maps, _, unscramble = build_all(x, edge_index, W, b, prelu_a)
    res = run_bass_kernel_spmd(nc, in_maps, core_ids=list(range(N_CORES)))
    y = np.concatenate(
        [np.asarray(res.results[c]["y"]) for c in range(N_CORES)], axis=0
    )
    return unscramble(y)
